# revision 1
# baseline (speedup 1.0000x reference)
"""Trainium2 Bass kernel for a binarized-weight MLP (BNN MNIST-style):

    h   = x @ sign(W1).T + b1      # fc1, binarized weights
    h   = clip(h, -1, 1)           # Hardtanh
    out = h @ W2.T + b2            # fc2

Shapes: x [8192, 784] f32, W1 [4096, 784], b1 [4096], W2 [10, 4096], b2 [10].

Strategy (data-parallel over 8 NeuronCores):
  - Shard batch 8192 -> 1024 rows/core; replicate weights.
  - All matmuls in bf16 (sign(W1) in {-1,0,+1} is exact in bf16), fp32 PSUM.
  - Bias folding: append ones-rows to x^T and put b1 (hi+lo bf16 split) as
    extra rows of the fc1 weight, so fc1 bias costs nothing. K = 784+2
    zero-padded to 896 = 7 k-tiles of 128.
  - fc1 computes h^T tiles [128 hid, 512 batch]; DVE tensor_scalar(min 1,
    max -1) applies Hardtanh and casts to bf16; fc2 accumulates
    W2^T (k-tiles [128,10]) @ h^T into a [10, 512] PSUM tile, software-
    pipelined one ht iteration behind fc1 to hide DVE latency.
  - Per-core output is out^T [10, 1024] f32; host gathers + transposes.
"""

import numpy as np
import ml_dtypes
from contextlib import ExitStack

import concourse.bass as bass
import concourse.mybir as mybir
import concourse.tile as tile
from concourse import bacc
from concourse import bass_utils

BF16_NP = ml_dtypes.bfloat16
BF16 = mybir.dt.bfloat16
F32 = mybir.dt.float32

BATCH, IN, HID, OUT = 8192, 784, 4096, 10
NCORES = 8
B_CORE = BATCH // NCORES        # 1024
NT = B_CORE // 512              # 2 batch n-tiles of 512 per core
HT = HID // 128                 # 32 hidden tiles
KT = 7                          # ceil((784+2)/128) k-tiles
K_PAD = KT * 128                # 896
N_WARMUP = 2                    # PE warm-up matmuls (HAM un-throttle)
UNPAIRED_HEAD = 2               # ht groups that run nt=0 only at the start

_CACHE = {}


def _build():
    """Build + compile the Bacc graph once per process."""
    if "nc" in _CACHE:
        return _CACHE["nc"]

    nc = bacc.Bacc("TRN2", target_bir_lowering=False, debug=False,
                   num_devices=NCORES)
    xt_d = nc.dram_tensor("xt", [NT, KT, 128, 512], BF16,
                          kind="ExternalInput").ap()
    w1_d = nc.dram_tensor("w1", [HT, 128, K_PAD], BF16,
                          kind="ExternalInput").ap()
    w2_d = nc.dram_tensor("w2", [128, HT * OUT], BF16,
                          kind="ExternalInput").ap()
    b2_d = nc.dram_tensor("b2", [OUT, 1], F32, kind="ExternalInput").ap()
    out_d = nc.dram_tensor("out", [OUT, B_CORE], F32,
                           kind="ExternalOutput").ap()

    # Raw (non-Tile) SBUF tensor for PE warm-up matmuls: contents are
    # irrelevant, so reading it uninitialized is fine and needs no producer.
    warm_sb = nc.alloc_sbuf_tensor("warm_raw", [128, 512], BF16).ap()

    with tile.TileContext(nc) as tc:
        with ExitStack() as ctx:
            wpool = ctx.enter_context(tc.tile_pool(name="w1", bufs=1))
            xpool = ctx.enter_context(tc.tile_pool(name="x", bufs=1))
            cpool = ctx.enter_context(tc.tile_pool(name="const", bufs=1))
            hpool = ctx.enter_context(tc.tile_pool(name="h", bufs=3))
            ps1pool = ctx.enter_context(
                tc.tile_pool(name="ps1", bufs=2, space="PSUM"))
            ps2pool = ctx.enter_context(
                tc.tile_pool(name="ps2", bufs=1, space="PSUM"))

            w2_sb = cpool.tile([128, HT * OUT], BF16, tag="w2")
            b2_sb = cpool.tile([OUT, 1], F32, tag="b2")
            out_sb = cpool.tile([OUT, B_CORE], F32, tag="out")

            # PE warm-up: the HAM clock gate keeps the PE at 1.2 GHz until
            # ~3.4us of sustained matmul activity. Run dummy matmuls while
            # input DMAs stream so real matmuls start at 2.4 GHz. warm_sb is
            # uninitialized (garbage bf16 is fine; results are discarded) so
            # the warm-ups have no dependencies and start right after init.
            wspool = ctx.enter_context(
                tc.tile_pool(name="pswarm", bufs=1, space="PSUM"))
            for i in range(N_WARMUP):
                pw = wspool.tile([128, 512], F32, tag="pswarm")
                nc.tensor.matmul(pw[:], warm_sb[:, 0:128], warm_sb[:],
                                 start=True, stop=True)

            # Input DMAs on two parallel HWDGE queues (sync + scalar),
            # interleaved so the first-consumed tiles land first:
            #   sync:   w1[0], x(0/1, odd kt), w1[1..31]
            #   scalar: x(0/1, even kt), w2, b2
            x_t = {}
            w1_t = []

            def w1_dma(ht):
                t = wpool.tile([128, K_PAD], BF16, tag=f"w1_{ht}",
                               name=f"w1_{ht}")
                nc.sync.dma_start(t[:], w1_d[ht])
                w1_t.append(t)

            def x_dma(nt, kt, eng):
                t = xpool.tile([128, 512], BF16, tag=f"x_{nt}_{kt}")
                eng.dma_start(t[:], xt_d[nt, kt])
                x_t[(nt, kt)] = t

            w1_dma(0)
            for kt in (1, 3):
                x_dma(0, kt, nc.sync)
            w1_dma(1)
            w1_dma(2)
            for kt in (1, 3, 5):
                x_dma(1, kt, nc.sync)
            for ht in range(3, HT):
                w1_dma(ht)
            for kt in (0, 2, 4, 6, 5):
                x_dma(0, kt, nc.scalar)
            for kt in (0, 2, 4, 6):
                x_dma(1, kt, nc.scalar)
            nc.scalar.dma_start(w2_sb[:], w2_d)
            nc.scalar.dma_start(b2_sb[:], b2_d)

            # fc1 iterates (ht, kt, nt): the two nt matmuls share the same
            # stationary lhsT back-to-back (faster weight path). The first
            # UNPAIRED_HEAD ht groups run nt=0 only — they are DMA-paced and
            # this halves the x bytes needed early; their nt=1 halves run at
            # the end when everything is resident. fc2 matmuls are pipelined
            # one group behind so the PE never waits on the DVE eviction.
            ps2 = [ps2pool.tile([OUT, 512], F32, tag=f"ps2_{nt}",
                                name=f"ps2_{nt}")
                   for nt in range(NT)]
            pending = []

            groups = ([(ht, (0,)) for ht in range(UNPAIRED_HEAD)]
                      + [(ht, (0, 1)) for ht in range(UNPAIRED_HEAD, HT)]
                      + [(ht, (1,)) for ht in range(UNPAIRED_HEAD)])
            fc2_first = {0: 0, 1: UNPAIRED_HEAD % HT}
            fc2_last = {0: HT - 1, 1: (UNPAIRED_HEAD - 1) % HT}

            def fc2_mm(pht, pnt, ph):
                nc.tensor.matmul(
                    ps2[pnt][:], w2_sb[:, pht * OUT:(pht + 1) * OUT], ph[:],
                    start=(pht == fc2_first[pnt]),
                    stop=(pht == fc2_last[pnt]),
                    skip_group_check=True)

            for ht, nts in groups:
                ps1 = {nt: ps1pool.tile([128, 512], F32, tag=f"ps1_{nt}",
                                        name=f"ps1_{ht}_{nt}")
                       for nt in nts}
                # group 0 consumes x(0,kt) in DMA-arrival order
                kt_order = (0, 1, 2, 3, 4, 6, 5) if ht == 0 and nts == (0,) \
                    else tuple(range(KT))
                for i, kt in enumerate(kt_order):
                    for nt in nts:
                        nc.tensor.matmul(
                            ps1[nt][:],
                            w1_t[ht][:, kt * 128:(kt + 1) * 128],
                            x_t[(nt, kt)][:],
                            start=(i == 0), stop=(i == KT - 1),
                            skip_group_check=True)
                for nt in nts:
                    h = hpool.tile([128, 512], BF16, tag=f"h_{nt}")
                    # Hardtanh + downcast: h = max(min(ps1, 1), -1)
                    nc.vector.tensor_scalar(
                        h[:], ps1[nt][:], 1.0, -1.0,
                        op0=mybir.AluOpType.min, op1=mybir.AluOpType.max)
                    pending.append((ht, nt, h))
                while len(pending) > NT:
                    fc2_mm(*pending.pop(0))
            for pht, pnt, ph in pending:
                fc2_mm(pht, pnt, ph)
            for nt in range(NT):
                # out = ps2 + b2 (per-partition bias), f32
                nc.scalar.activation(
                    out_sb[:, nt * 512:(nt + 1) * 512], ps2[nt][:],
                    mybir.ActivationFunctionType.Identity, bias=b2_sb[:])
            nc.sync.dma_start(out_d, out_sb[:])

    nc.compile()
    _CACHE["nc"] = nc
    return nc


def _prep_inputs(x, W1, b1, W2, b2):
    """Host-side shard + layout prep. Returns in_maps for the 8 cores."""
    x = np.asarray(x, dtype=np.float32)
    W1 = np.asarray(W1, dtype=np.float32)
    b1 = np.asarray(b1, dtype=np.float32)
    W2 = np.asarray(W2, dtype=np.float32)
    b2 = np.asarray(b2, dtype=np.float32)

    # fc1 weight, augmented with two bias rows (hi + lo bf16 split of b1),
    # zero-padded to K_PAD. Layout [ht, p, kt*128+m] = w1aug[kt*128+p, ht*128+m].
    w1aug = np.zeros((K_PAD, HID), dtype=np.float32)
    w1aug[:IN] = np.sign(W1).T
    b1_hi = b1.astype(BF16_NP).astype(np.float32)
    w1aug[IN] = b1_hi
    w1aug[IN + 1] = b1 - b1_hi
    w1_host = np.ascontiguousarray(
        w1aug.astype(BF16_NP).reshape(KT, 128, HT, 128)
        .transpose(2, 1, 0, 3).reshape(HT, 128, K_PAD))

    # fc2 weight: [p, kt*10+o] = W2[o, kt*128+p]
    w2_host = np.ascontiguousarray(
        W2.T.astype(BF16_NP).reshape(HT, 128, OUT)
        .transpose(1, 0, 2).reshape(128, HT * OUT))

    b2_host = np.ascontiguousarray(b2.reshape(OUT, 1))

    # x augmented with ones-columns matching the two b1 rows.
    x_aug = np.zeros((BATCH, K_PAD), dtype=BF16_NP)
    x_aug[:, :IN] = x.astype(BF16_NP)
    x_aug[:, IN] = 1
    x_aug[:, IN + 1] = 1

    in_maps = []
    for c in range(NCORES):
        xc = x_aug[c * B_CORE:(c + 1) * B_CORE]          # [1024, 896]
        xt = np.ascontiguousarray(
            xc.reshape(NT, 512, KT, 128).transpose(0, 2, 3, 1))
        in_maps.append({"xt": xt, "w1": w1_host, "w2": w2_host,
                        "b2": b2_host})
    return in_maps


def _gather(results):
    full = np.concatenate([np.asarray(r["out"], dtype=np.float32)
                           for r in results], axis=1)    # [10, 8192]
    return np.ascontiguousarray(full.T)                  # [8192, 10]


def run(x, W1, b1, W2, b2, trace=False, **trace_kwargs):
    import os
    nc = _build()
    in_maps = _prep_inputs(x, W1, b1, W2, b2)
    if not trace:
        # The NTFF profiling hook isn't available in every environment;
        # make sure an ambient BASS_TRACE can't pull us onto that path.
        os.environ["BASS_NEVER_TRACE"] = "1"
    else:
        os.environ.pop("BASS_NEVER_TRACE", None)
    res = bass_utils.run_bass_kernel_spmd(
        nc, in_maps, core_ids=list(range(NCORES)), trace=trace,
        **trace_kwargs)
    return _gather(res.results), res


def kernel(x, W1, b1, W2, b2):
    out, _ = run(x, W1, b1, W2, b2)
    return out



# revision 13
# speedup vs baseline: 1.1341x; 1.1341x over previous
"""Trainium2 Bass kernel for a binarized-weight MLP (BNN MNIST-style):

    h   = x @ sign(W1).T + b1      # fc1, binarized weights
    h   = clip(h, -1, 1)           # Hardtanh
    out = h @ W2.T + b2            # fc2

Shapes: x [8192, 784] f32, W1 [4096, 784], b1 [4096], W2 [10, 4096], b2 [10].

Strategy (data-parallel over 8 NeuronCores):
  - Shard batch 8192 -> 1024 rows/core; replicate weights. All matmuls bf16
    (sign(W1) exact in bf16), fp32 PSUM.
  - fc1 contraction K = 784 x-rows + 2 bias rows (b1 hi/lo bf16 split with
    ones-columns in x) = 786 rows = 6 full 128-row k-tiles + an 18-row tail.
  - The 18-row tail is ROW-TILED: hidden tiles are processed in quads of 4,
    and one PE slot runs 4 concurrent K=18 matmuls at tile_position
    (0/32/64/96, 0) - one per ht in the quad - so the tail costs 1 slot per
    quad instead of 4.  Slot order per quad: packed tail first (start=True
    for all 4 PSUM groups), then ht-by-ht k0..5 with stop at k5, so each
    ht's Hardtanh (DVE tensor_scalar min/max) pipelines inside the quad.
  - fc2 (M=10) is COL-TILED: per quad one burst of 4 concurrent matmuls at
    tile_position (0, 0/32/64/96) accumulates W2-slices @ h into partition
    slices 32r..32r+9 of a per-nt PSUM tile; bursts are issued one quad
    behind fc1.  The 4 partition groups are reduced by one selector matmul
    (sel[32r+o, o] = 1) over an ACT-engine copy of the PSUM tile, with b2
    folded into the selector via a ones-row at partition 127.
  - 4 PE warm-up matmuls bridge the DMA head so the HAM clock gate
    un-throttles (1.2 -> 2.4 GHz) as early as possible.
  - Per-core output is out^T [10, 1024] f32; host gathers + transposes.
"""

import numpy as np
import ml_dtypes
from contextlib import ExitStack

import concourse.bass as bass
import concourse.mybir as mybir
import concourse.tile as tile
from concourse import bacc
from concourse import bass_utils

BF16_NP = ml_dtypes.bfloat16
BF16 = mybir.dt.bfloat16
F32 = mybir.dt.float32

BATCH, IN, HID, OUT = 8192, 784, 4096, 10
NCORES = 8
B_CORE = BATCH // NCORES        # 1024
NT = B_CORE // 512              # 2 batch n-tiles of 512 per core
HT = HID // 128                 # 32 hidden tiles
NQ = HT // 4                    # 8 quads of 4 hidden tiles
KF = 6                          # full 128-row k-tiles (768 rows)
KTAIL = IN + 2 - KF * 128       # 18 tail rows (16 x-rows + 2 bias rows)
N_WARMUP = 4                    # PE warm-up matmuls (HAM un-throttle)

_CACHE = {}


def _build():
    """Build + compile the Bacc graph once per process."""
    if "nc" in _CACHE:
        return _CACHE["nc"]

    nc = bacc.Bacc("TRN2", target_bir_lowering=False, debug=False,
                   num_devices=NCORES)
    xt_d = nc.dram_tensor("xt", [NT, KF, 128, 512], BF16,
                          kind="ExternalInput").ap()
    xt6_d = nc.dram_tensor("xt6", [NT, 128, 512], BF16,
                           kind="ExternalInput").ap()
    w1_d = nc.dram_tensor("w1", [HT, 128, KF * 128], BF16,
                          kind="ExternalInput").ap()
    w1k7_d = nc.dram_tensor("w1k7", [128, NQ * 128], BF16,
                            kind="ExternalInput").ap()
    w2_d = nc.dram_tensor("w2", [128, HT * OUT], BF16,
                          kind="ExternalInput").ap()
    sel_d = nc.dram_tensor("sel", [128, OUT], BF16,
                           kind="ExternalInput").ap()
    out_d = nc.dram_tensor("out", [OUT, B_CORE], F32,
                           kind="ExternalOutput").ap()

    # Raw SBUF tensor for PE warm-up matmuls: contents irrelevant, results
    # discarded into a scratch PSUM bank that is later reused by the
    # selector matmul.
    warm_sb = nc.alloc_sbuf_tensor("warm_raw", [128, 512], BF16).ap()

    with tile.TileContext(nc) as tc:
        with ExitStack() as ctx:
            wpool = ctx.enter_context(tc.tile_pool(name="w1", bufs=1))
            xpool = ctx.enter_context(tc.tile_pool(name="x", bufs=1))
            cpool = ctx.enter_context(tc.tile_pool(name="const", bufs=1))
            # h: single rotating tag; 9 bufs >> the ~5-alloc read distance
            hpool = ctx.enter_context(tc.tile_pool(name="h", bufs=9))
            s2pool = ctx.enter_context(tc.tile_pool(name="s2", bufs=1))
            # PSUM bank budget (8 banks of [128,512]f32):
            #   ps1 x5 (rotating; 5 > 4/quad so a quad's packed matmuls
            #   never WAR-wait on the previous quad's in-flight Hardtanh
            #   read of the same bank) + ps2 x2 (per nt) + psel/warm x1
            ps1pool = ctx.enter_context(
                tc.tile_pool(name="ps1", bufs=5, space="PSUM"))
            ps2pool = ctx.enter_context(
                tc.tile_pool(name="ps2", bufs=1, space="PSUM"))
            pselpool = ctx.enter_context(
                tc.tile_pool(name="psel", bufs=1, space="PSUM"))

            w1k7_sb = cpool.tile([128, NQ * 128], BF16, tag="w1k7")
            w2_sb = cpool.tile([128, HT * OUT], BF16, tag="w2")
            sel_sb = cpool.tile([128, OUT], BF16, tag="sel")
            out_sb = cpool.tile([OUT, B_CORE], F32, tag="out")
            xt6_sb = [cpool.tile([128, 512], BF16, tag=f"xt6_{nt}",
                                 name=f"xt6_{nt}")
                      for nt in range(NT)]

            # PE warm-up: HAM keeps the PE at 1.2 GHz until ~3.4us of
            # sustained matmul activity; run dummy matmuls (no deps, start
            # immediately) while the first input DMAs stream.  They share
            # the psel PSUM bank (long free before the selector needs it).
            for i in range(N_WARMUP):
                pw = pselpool.tile([128, 512], F32, tag="psel",
                                   name=f"pswarm_{i}")
                nc.tensor.matmul(pw[:], warm_sb[:, 0:128], warm_sb[:],
                                 start=True, stop=True)

            # Input DMAs on two parallel HWDGE queues, ordered so the
            # first-consumed tiles land first:
            #   sync:   w1k7, w1[0..31]
            #   scalar: xt6[0], x(0,k*), w2, sel, xt6[1], x(1,k*)
            x_t = {}
            w1_t = []

            nc.sync.dma_start(w1k7_sb[:], w1k7_d)
            for ht in range(HT):
                t = wpool.tile([128, KF * 128], BF16, tag=f"w1_{ht}",
                               name=f"w1_{ht}")
                nc.sync.dma_start(t[:], w1_d[ht])
                w1_t.append(t)

            nc.scalar.dma_start(xt6_sb[0][:], xt6_d[0])
            for kt in range(KF):
                t = xpool.tile([128, 512], BF16, tag=f"x_0_{kt}")
                nc.scalar.dma_start(t[:], xt_d[0, kt])
                x_t[(0, kt)] = t
            nc.scalar.dma_start(w2_sb[:], w2_d)
            nc.scalar.dma_start(sel_sb[:], sel_d)
            nc.scalar.dma_start(xt6_sb[1][:], xt6_d[1])
            for kt in range(KF):
                t = xpool.tile([128, 512], BF16, tag=f"x_1_{kt}")
                nc.scalar.dma_start(t[:], xt_d[1, kt])
                x_t[(1, kt)] = t

            # deferred actions, emitted between fc1 ht-groups so PE-queue
            # stalls on not-yet-ready DVE/ACT results are avoided
            deferred = []

            def fc2_burst(nt, q, hs):
                def go():
                    for r in range(4):
                        ht = 4 * q + r
                        nc.tensor.matmul(
                            ps2[nt][32 * r:32 * r + OUT, :],
                            w2_sb[:, ht * OUT:(ht + 1) * OUT], hs[r][:],
                            start=(q == 0), stop=(q == NQ - 1),
                            tile_position=(0, 32 * r),
                            skip_group_check=True)
                return go

            def sel_chain(nt):
                # ps2[nt] partition groups {0,32,64,96}+0..9 -> out via one
                # selector matmul; b2 rides on sel partition 127 against the
                # ones rows of s2 (pre-memset to 1.0 at kernel start).
                s2 = s2_sb[nt]
                psel = pselpool.tile([128, 512], F32, tag="psel",
                                     name=f"psel_{nt}")

                def copy():
                    nc.scalar.activation(
                        s2[0:106, :], ps2[nt][0:106, :],
                        mybir.ActivationFunctionType.Identity)

                def mm():
                    nc.tensor.matmul(psel[0:OUT, :], sel_sb[:], s2[:],
                                     start=True, stop=True,
                                     skip_group_check=True)

                def out():
                    nc.scalar.activation(
                        out_sb[:, nt * 512:(nt + 1) * 512], psel[0:OUT, :],
                        mybir.ActivationFunctionType.Identity)
                    nc.sync.dma_start(
                        out_d[:, nt * 512:(nt + 1) * 512],
                        out_sb[:, nt * 512:(nt + 1) * 512])

                def noop():
                    pass
                return [copy, noop, mm, out]

            ps2 = [ps2pool.tile([128, 512], F32, tag=f"ps2_{nt}",
                                name=f"ps2_{nt}")
                   for nt in range(NT)]
            s2_sb = [s2pool.tile([128, 512], BF16, tag=f"s2_{nt}",
                                 name=f"s2_{nt}")
                     for nt in range(NT)]
            # fc2 only ever writes partition groups 32r..32r+9 of ps2, and
            # the ACT copy in sel_chain reads ps2[0:106) into s2 whose rows
            # 106..127 must be 1.0 (bias row) — stale PSUM/SBUF could hold
            # NaN and NaN*0 = NaN in the selector matmul.  Full-tile
            # memsets at kernel start (engines idle; start=True matmuls
            # overwrite their elements regardless).
            for nt in range(NT):
                nc.vector.memset(ps2[nt][:], 0.0)
                nc.gpsimd.memset(s2_sb[nt][:], 1.0)

            for nt in range(NT):
                for q in range(NQ):
                    ps1 = [ps1pool.tile([128, 512], F32, tag="ps1",
                                        name=f"ps1_{nt}_{q}_{r}")
                           for r in range(4)]
                    # packed 18-row tail: 4 concurrent row-tiled matmuls
                    for r in range(4):
                        nc.tensor.matmul(
                            ps1[r][:],
                            w1k7_sb[32 * r:32 * r + KTAIL,
                                    q * 128:(q + 1) * 128],
                            xt6_sb[nt][32 * r:32 * r + KTAIL, :],
                            start=True, stop=False,
                            tile_position=(32 * r, 0),
                            skip_group_check=True)
                    hs = []
                    for r in range(4):
                        ht = 4 * q + r
                        for kt in range(KF):
                            nc.tensor.matmul(
                                ps1[r][:],
                                w1_t[ht][:, kt * 128:(kt + 1) * 128],
                                x_t[(nt, kt)][:],
                                start=False, stop=(kt == KF - 1),
                                skip_group_check=True)
                        h = hpool.tile([128, 512], BF16, tag="h",
                                       name=f"h_{nt}_{ht}")
                        # Hardtanh + downcast: h = max(min(ps1, 1), -1)
                        nc.vector.tensor_scalar(
                            h[:], ps1[r][:], 1.0, -1.0,
                            op0=mybir.AluOpType.min,
                            op1=mybir.AluOpType.max)
                        hs.append(h)
                        # emit one deferred action after each ht group
                        if deferred:
                            deferred.pop(0)()
                    deferred.append(fc2_burst(nt, q, hs))
                    if q == NQ - 1:
                        deferred.extend(sel_chain(nt))
            for fn in deferred:
                fn()

    nc.compile()
    _CACHE["nc"] = nc
    return nc


def _prep_inputs(x, W1, b1, W2, b2):
    """Host-side shard + layout prep. Returns in_maps for the 8 cores."""
    x = np.asarray(x, dtype=np.float32)
    W1 = np.asarray(W1, dtype=np.float32)
    b1 = np.asarray(b1, dtype=np.float32)
    W2 = np.asarray(W2, dtype=np.float32)
    b2 = np.asarray(b2, dtype=np.float32)

    K = IN + 2                                           # 786
    # fc1 weight, augmented with two bias rows (hi + lo bf16 split of b1).
    w1aug = np.zeros((K, HID), dtype=np.float32)
    w1aug[:IN] = np.sign(W1).T
    b1_hi = b1.astype(BF16_NP).astype(np.float32)
    w1aug[IN] = b1_hi
    w1aug[IN + 1] = b1 - b1_hi
    w1aug = w1aug.astype(BF16_NP)

    # full k-tiles: [ht, p, kt*128+m] = w1aug[kt*128+p, ht*128+m]
    w1_host = np.ascontiguousarray(
        w1aug[:KF * 128].reshape(KF, 128, HT, 128)
        .transpose(2, 1, 0, 3).reshape(HT, 128, KF * 128))

    # 18-row tail, packed for 4-way row tiling:
    # [32r+j, q*128+m] = w1aug[768+j, (4q+r)*128+m]
    w1k7 = np.zeros((128, NQ * 128), dtype=BF16_NP)
    tail = w1aug[KF * 128:].reshape(KTAIL, NQ, 4, 128)   # [j, q, r, m]
    for r in range(4):
        w1k7[32 * r:32 * r + KTAIL] = tail[:, :, r, :].reshape(KTAIL, -1)

    # fc2 weight: [p, ht*10+o] = W2[o, ht*128+p]
    w2_host = np.ascontiguousarray(
        W2.T.astype(BF16_NP).reshape(HT, 128, OUT)
        .transpose(1, 0, 2).reshape(128, HT * OUT))

    # selector for the 4-way fc2 partition-group reduction, b2 on row 127
    sel_host = np.zeros((128, OUT), dtype=BF16_NP)
    for r in range(4):
        for o in range(OUT):
            sel_host[32 * r + o, o] = 1
    sel_host[127, :] = b2.astype(BF16_NP)

    # x augmented with ones-columns matching the two b1 rows.
    x_aug = np.zeros((BATCH, K), dtype=BF16_NP)
    x_aug[:, :IN] = x.astype(BF16_NP)
    x_aug[:, IN] = 1
    x_aug[:, IN + 1] = 1

    in_maps = []
    for c in range(NCORES):
        xc = x_aug[c * B_CORE:(c + 1) * B_CORE]          # [1024, 786]
        # full k-tiles: [nt, kt, p, b] = xc[nt*512+b, kt*128+p]
        xt = np.ascontiguousarray(
            xc[:, :KF * 128].reshape(NT, 512, KF, 128).transpose(0, 2, 3, 1))
        # tail block replicated at partition bases 0/32/64/96
        xt6 = np.zeros((NT, 128, 512), dtype=BF16_NP)
        tail_x = (xc[:, KF * 128:].reshape(NT, 512, KTAIL)
                  .transpose(0, 2, 1))                   # [nt, j, b]
        for r in range(4):
            xt6[:, 32 * r:32 * r + KTAIL, :] = tail_x
        in_maps.append({"xt": xt, "xt6": xt6, "w1": w1_host,
                        "w1k7": w1k7, "w2": w2_host, "sel": sel_host})
    return in_maps


def _gather(results):
    full = np.concatenate([np.asarray(r["out"], dtype=np.float32)
                           for r in results], axis=1)    # [10, 8192]
    return np.ascontiguousarray(full.T)                  # [8192, 10]


def run(x, W1, b1, W2, b2, trace=False, **trace_kwargs):
    import os
    nc = _build()
    in_maps = _prep_inputs(x, W1, b1, W2, b2)
    if not trace:
        # The NTFF profiling hook isn't available in every environment;
        # make sure an ambient BASS_TRACE can't pull us onto that path.
        os.environ["BASS_NEVER_TRACE"] = "1"
    else:
        os.environ.pop("BASS_NEVER_TRACE", None)
    res = bass_utils.run_bass_kernel_spmd(
        nc, in_maps, core_ids=list(range(NCORES)), trace=trace,
        **trace_kwargs)
    return _gather(res.results), res


def kernel(x, W1, b1, W2, b2):
    out, _ = run(x, W1, b1, W2, b2)
    return out


# revision 16
# speedup vs baseline: 1.1490x; 1.0132x over previous
"""Trainium2 Bass kernel for a binarized-weight MLP (BNN MNIST-style):

    h   = x @ sign(W1).T + b1      # fc1, binarized weights
    h   = clip(h, -1, 1)           # Hardtanh
    out = h @ W2.T + b2            # fc2

Shapes: x [8192, 784] f32, W1 [4096, 784], b1 [4096], W2 [10, 4096], b2 [10].

Strategy (data-parallel over 8 NeuronCores):
  - Shard batch 8192 -> 1024 rows/core; replicate weights. All matmuls bf16
    (sign(W1) exact in bf16), fp32 PSUM.
  - fc1 contraction K = 784 x-rows + 2 bias rows (b1 hi/lo bf16 split with
    ones-columns in x) = 786 rows = 6 full 128-row k-tiles + an 18-row tail.
  - The 18-row tail is ROW-TILED: hidden tiles are processed in quads of 4,
    and one PE slot runs 4 concurrent K=18 matmuls at tile_position
    (0/32/64/96, 0) - one per ht in the quad - so the tail costs 1 slot per
    quad instead of 4.  Slot order per quad: packed tail first (start=True
    for all 4 PSUM groups), then ht-by-ht k0..5 with stop at k5, so each
    ht's Hardtanh (DVE tensor_scalar min/max) pipelines inside the quad.
  - fc2 (M=10) is COL-TILED: per quad one burst of 4 concurrent matmuls at
    tile_position (0, 0/32/64/96) accumulates W2-slices @ h into partition
    slices 32r..32r+9 of a per-nt PSUM tile; bursts are issued one quad
    behind fc1.  The 4 partition groups are reduced by one selector matmul
    (sel[32r+o, o] = 1) over an ACT-engine copy of the PSUM tile, with b2
    folded into the selector via a ones-row at partition 127.
  - 4 PE warm-up matmuls bridge the DMA head so the HAM clock gate
    un-throttles (1.2 -> 2.4 GHz) as early as possible.
  - Per-core output is out^T [10, 1024] f32; host gathers + transposes.
"""

import numpy as np
import ml_dtypes
from contextlib import ExitStack

import concourse.bass as bass
import concourse.mybir as mybir
import concourse.tile as tile
from concourse import bacc
from concourse import bass_utils

BF16_NP = ml_dtypes.bfloat16
BF16 = mybir.dt.bfloat16
F32 = mybir.dt.float32

BATCH, IN, HID, OUT = 8192, 784, 4096, 10
NCORES = 8
B_CORE = BATCH // NCORES        # 1024
NT = B_CORE // 512              # 2 batch n-tiles of 512 per core
HT = HID // 128                 # 32 hidden tiles
NQ = HT // 4                    # 8 quads of 4 hidden tiles
KF = 6                          # full 128-row k-tiles (768 rows)
KTAIL = IN + 2 - KF * 128       # 18 tail rows (16 x-rows + 2 bias rows)
N_WARMUP = 4                    # PE warm-up matmuls (HAM un-throttle)

_CACHE = {}


def _build():
    """Build + compile the Bacc graph once per process."""
    if "nc" in _CACHE:
        return _CACHE["nc"]

    nc = bacc.Bacc("TRN2", target_bir_lowering=False, debug=False,
                   num_devices=NCORES)
    xt_d = nc.dram_tensor("xt", [NT, KF, 128, 512], BF16,
                          kind="ExternalInput").ap()
    xt6_d = nc.dram_tensor("xt6", [NT, 128, 512], BF16,
                           kind="ExternalInput").ap()
    w1_d = nc.dram_tensor("w1", [HT, 128, KF * 128], BF16,
                          kind="ExternalInput").ap()
    w1k7_d = nc.dram_tensor("w1k7", [128, NQ * 128], BF16,
                            kind="ExternalInput").ap()
    w2_d = nc.dram_tensor("w2", [128, HT * OUT], BF16,
                          kind="ExternalInput").ap()
    sel_d = nc.dram_tensor("sel", [128, OUT], BF16,
                           kind="ExternalInput").ap()
    out_d = nc.dram_tensor("out", [OUT, B_CORE], F32,
                           kind="ExternalOutput").ap()

    # Raw SBUF tensor for PE warm-up matmuls: contents irrelevant, results
    # discarded into a scratch PSUM bank that is later reused by the
    # selector matmul.
    warm_sb = nc.alloc_sbuf_tensor("warm_raw", [128, 512], BF16).ap()

    with tile.TileContext(nc) as tc:
        with ExitStack() as ctx:
            wpool = ctx.enter_context(tc.tile_pool(name="w1", bufs=1))
            xpool = ctx.enter_context(tc.tile_pool(name="x", bufs=1))
            cpool = ctx.enter_context(tc.tile_pool(name="const", bufs=1))
            # h: single rotating tag; 13 bufs >> the ~9-alloc read distance
            # (fc2 bursts are batched every 2 quads)
            hpool = ctx.enter_context(tc.tile_pool(name="h", bufs=13))
            s2pool = ctx.enter_context(tc.tile_pool(name="s2", bufs=1))
            # PSUM bank budget (8 banks of [128,512]f32):
            #   ps1 x6 (rotating; 6 > 4/quad + 1 so a quad's packed matmuls
            #   never WAR-wait on the previous quad's in-flight Hardtanh
            #   read of the same bank) + ps2 x1 (shared by both nt; nt0's
            #   is ACT-drained before nt1's first deferred burst) +
            #   psel/warm x1
            ps1pool = ctx.enter_context(
                tc.tile_pool(name="ps1", bufs=6, space="PSUM"))
            ps2pool = ctx.enter_context(
                tc.tile_pool(name="ps2", bufs=1, space="PSUM"))
            pselpool = ctx.enter_context(
                tc.tile_pool(name="psel", bufs=1, space="PSUM"))

            w1k7_sb = cpool.tile([128, NQ * 128], BF16, tag="w1k7")
            w2_sb = cpool.tile([128, HT * OUT], BF16, tag="w2")
            sel_sb = cpool.tile([128, OUT], BF16, tag="sel")
            out_sb = cpool.tile([OUT, B_CORE], F32, tag="out")
            xt6_sb = [cpool.tile([128, 512], BF16, tag=f"xt6_{nt}",
                                 name=f"xt6_{nt}")
                      for nt in range(NT)]

            # PE warm-up: HAM keeps the PE at 1.2 GHz until ~3.4us of
            # sustained matmul activity; run dummy matmuls (no deps, start
            # immediately) while the first input DMAs stream.  They share
            # the psel PSUM bank (long free before the selector needs it).
            for i in range(N_WARMUP):
                pw = pselpool.tile([128, 512], F32, tag="psel",
                                   name=f"pswarm_{i}")
                nc.tensor.matmul(pw[:], warm_sb[:, 0:128], warm_sb[:],
                                 start=True, stop=True)

            # Input DMAs on two parallel HWDGE queues, ordered so the
            # first-consumed tiles land first:
            #   sync:   w1k7, w1[0..31]
            #   scalar: xt6[0], x(0,k*), w2, sel, xt6[1], x(1,k*)
            x_t = {}
            w1_t = [wpool.tile([128, KF * 128], BF16, tag=f"w1_{ht}",
                               name=f"w1_{ht}")
                    for ht in range(HT)]
            for nt in range(NT):
                for kt in range(KF):
                    x_t[(nt, kt)] = xpool.tile([128, 512], BF16,
                                               tag=f"x_{nt}_{kt}",
                                               name=f"x_{nt}_{kt}")

            # sync feeds quad 0's critical path first (w1k7 slice for q0,
            # w1[0], the last two x tiles of nt0), then streams w1.
            nc.sync.dma_start(w1k7_sb[:, 0:128], w1k7_d[:, 0:128])
            nc.sync.dma_start(w1_t[0][:], w1_d[0])
            for kt in (4, 5):
                nc.sync.dma_start(x_t[(0, kt)][:], xt_d[0, kt])
            nc.sync.dma_start(w1k7_sb[:, 128:NQ * 128],
                              w1k7_d[:, 128:NQ * 128])
            for ht in range(1, HT):
                nc.sync.dma_start(w1_t[ht][:], w1_d[ht])

            nc.scalar.dma_start(xt6_sb[0][:], xt6_d[0])
            for kt in range(4):
                nc.scalar.dma_start(x_t[(0, kt)][:], xt_d[0, kt])
            nc.scalar.dma_start(w2_sb[:], w2_d)
            nc.scalar.dma_start(sel_sb[:], sel_d)
            nc.scalar.dma_start(xt6_sb[1][:], xt6_d[1])
            for kt in range(KF):
                nc.scalar.dma_start(x_t[(1, kt)][:], xt_d[1, kt])

            # deferred actions, emitted between fc1 ht-groups so PE-queue
            # stalls on not-yet-ready DVE/ACT results are avoided
            deferred = []

            def sel_chain(nt):
                # ps2 partition groups {0,32,64,96}+0..9 -> out via one
                # selector matmul; b2 rides on sel partition 127 against the
                # ones rows of s2 (pre-memset to 1.0 at kernel start).
                s2 = s2_sb[nt]
                psel = pselpool.tile([128, 512], F32, tag="psel",
                                     name=f"psel_{nt}")

                def copy():
                    nc.scalar.activation(
                        s2[0:106, :], ps2[0:106, :],
                        mybir.ActivationFunctionType.Identity)

                def mm():
                    nc.tensor.matmul(psel[0:OUT, :], sel_sb[:], s2[:],
                                     start=True, stop=True,
                                     skip_group_check=True)

                def out():
                    nc.scalar.activation(
                        out_sb[:, nt * 512:(nt + 1) * 512], psel[0:OUT, :],
                        mybir.ActivationFunctionType.Identity)
                    nc.sync.dma_start(
                        out_d[:, nt * 512:(nt + 1) * 512],
                        out_sb[:, nt * 512:(nt + 1) * 512])

                def noop():
                    pass
                return [copy, noop, mm, out]

            ps2 = ps2pool.tile([128, 512], F32, tag="ps2", name="ps2")
            s2_sb = [s2pool.tile([128, 512], BF16, tag=f"s2_{nt}",
                                 name=f"s2_{nt}")
                     for nt in range(NT)]
            # fc2 only ever writes partition groups 32r..32r+9 of ps2, and
            # the ACT copy in sel_chain reads ps2[0:106) into s2 whose rows
            # 106..127 must be 1.0 (bias row) — stale PSUM/SBUF could hold
            # NaN and NaN*0 = NaN in the selector matmul.  Full-tile
            # memsets at kernel start (engines idle; start=True matmuls
            # overwrite their elements regardless).
            nc.vector.memset(ps2[:], 0.0)
            for nt in range(NT):
                nc.gpsimd.memset(s2_sb[nt][:], 1.0)

            # fc2 col-tiled matmuls, batched every other quad and emitted
            # right after a packed slot (all PE array-mask switches grouped
            # at the quad boundary).  Each burst runs 4 col groups
            # concurrently; within a group the (up to 2) quads serialize.
            pending_fc2 = []

            def flush_fc2():
                for r in range(4):
                    for q, hs in pending_fc2:
                        nc.tensor.matmul(
                            ps2[32 * r:32 * r + OUT, :],
                            w2_sb[:, (4 * q + r) * OUT:
                                  (4 * q + r + 1) * OUT], hs[r][:],
                            start=(q == 0), stop=(q == NQ - 1),
                            tile_position=(0, 32 * r),
                            skip_group_check=True)
                pending_fc2.clear()

            for nt in range(NT):
                for q in range(NQ):
                    ps1 = [ps1pool.tile([128, 512], F32, tag="ps1",
                                        name=f"ps1_{nt}_{q}_{r}")
                           for r in range(4)]
                    # packed 18-row tail: 4 concurrent row-tiled matmuls
                    for r in range(4):
                        nc.tensor.matmul(
                            ps1[r][:],
                            w1k7_sb[32 * r:32 * r + KTAIL,
                                    q * 128:(q + 1) * 128],
                            xt6_sb[nt][32 * r:32 * r + KTAIL, :],
                            start=True, stop=False,
                            tile_position=(32 * r, 0),
                            skip_group_check=True)
                    if len(pending_fc2) >= 2:
                        flush_fc2()
                    hs = []
                    for r in range(4):
                        ht = 4 * q + r
                        for kt in range(KF):
                            nc.tensor.matmul(
                                ps1[r][:],
                                w1_t[ht][:, kt * 128:(kt + 1) * 128],
                                x_t[(nt, kt)][:],
                                start=False, stop=(kt == KF - 1),
                                skip_group_check=True)
                        h = hpool.tile([128, 512], BF16, tag="h",
                                       name=f"h_{nt}_{ht}")
                        # Hardtanh + downcast: h = max(min(ps1, 1), -1)
                        nc.vector.tensor_scalar(
                            h[:], ps1[r][:], 1.0, -1.0,
                            op0=mybir.AluOpType.min,
                            op1=mybir.AluOpType.max)
                        hs.append(h)
                        # emit one deferred action after each ht group
                        if deferred:
                            deferred.pop(0)()
                    pending_fc2.append((q, hs))
                    if q == NQ - 1:
                        deferred.extend(sel_chain(nt))
            flush_fc2()
            for fn in deferred:
                fn()

    nc.compile()
    _CACHE["nc"] = nc
    return nc


def _prep_inputs(x, W1, b1, W2, b2):
    """Host-side shard + layout prep. Returns in_maps for the 8 cores."""
    x = np.asarray(x, dtype=np.float32)
    W1 = np.asarray(W1, dtype=np.float32)
    b1 = np.asarray(b1, dtype=np.float32)
    W2 = np.asarray(W2, dtype=np.float32)
    b2 = np.asarray(b2, dtype=np.float32)

    K = IN + 2                                           # 786
    # fc1 weight, augmented with two bias rows (hi + lo bf16 split of b1).
    w1aug = np.zeros((K, HID), dtype=np.float32)
    w1aug[:IN] = np.sign(W1).T
    b1_hi = b1.astype(BF16_NP).astype(np.float32)
    w1aug[IN] = b1_hi
    w1aug[IN + 1] = b1 - b1_hi
    w1aug = w1aug.astype(BF16_NP)

    # full k-tiles: [ht, p, kt*128+m] = w1aug[kt*128+p, ht*128+m]
    w1_host = np.ascontiguousarray(
        w1aug[:KF * 128].reshape(KF, 128, HT, 128)
        .transpose(2, 1, 0, 3).reshape(HT, 128, KF * 128))

    # 18-row tail, packed for 4-way row tiling:
    # [32r+j, q*128+m] = w1aug[768+j, (4q+r)*128+m]
    w1k7 = np.zeros((128, NQ * 128), dtype=BF16_NP)
    tail = w1aug[KF * 128:].reshape(KTAIL, NQ, 4, 128)   # [j, q, r, m]
    for r in range(4):
        w1k7[32 * r:32 * r + KTAIL] = tail[:, :, r, :].reshape(KTAIL, -1)

    # fc2 weight: [p, ht*10+o] = W2[o, ht*128+p]
    w2_host = np.ascontiguousarray(
        W2.T.astype(BF16_NP).reshape(HT, 128, OUT)
        .transpose(1, 0, 2).reshape(128, HT * OUT))

    # selector for the 4-way fc2 partition-group reduction, b2 on row 127
    sel_host = np.zeros((128, OUT), dtype=BF16_NP)
    for r in range(4):
        for o in range(OUT):
            sel_host[32 * r + o, o] = 1
    sel_host[127, :] = b2.astype(BF16_NP)

    # x augmented with ones-columns matching the two b1 rows.
    x_aug = np.zeros((BATCH, K), dtype=BF16_NP)
    x_aug[:, :IN] = x.astype(BF16_NP)
    x_aug[:, IN] = 1
    x_aug[:, IN + 1] = 1

    in_maps = []
    for c in range(NCORES):
        xc = x_aug[c * B_CORE:(c + 1) * B_CORE]          # [1024, 786]
        # full k-tiles: [nt, kt, p, b] = xc[nt*512+b, kt*128+p]
        xt = np.ascontiguousarray(
            xc[:, :KF * 128].reshape(NT, 512, KF, 128).transpose(0, 2, 3, 1))
        # tail block replicated at partition bases 0/32/64/96
        xt6 = np.zeros((NT, 128, 512), dtype=BF16_NP)
        tail_x = (xc[:, KF * 128:].reshape(NT, 512, KTAIL)
                  .transpose(0, 2, 1))                   # [nt, j, b]
        for r in range(4):
            xt6[:, 32 * r:32 * r + KTAIL, :] = tail_x
        in_maps.append({"xt": xt, "xt6": xt6, "w1": w1_host,
                        "w1k7": w1k7, "w2": w2_host, "sel": sel_host})
    return in_maps


def _gather(results):
    full = np.concatenate([np.asarray(r["out"], dtype=np.float32)
                           for r in results], axis=1)    # [10, 8192]
    return np.ascontiguousarray(full.T)                  # [8192, 10]


def run(x, W1, b1, W2, b2, trace=False, **trace_kwargs):
    import os
    nc = _build()
    in_maps = _prep_inputs(x, W1, b1, W2, b2)
    if not trace:
        # The NTFF profiling hook isn't available in every environment;
        # make sure an ambient BASS_TRACE can't pull us onto that path.
        os.environ["BASS_NEVER_TRACE"] = "1"
    else:
        os.environ.pop("BASS_NEVER_TRACE", None)
    res = bass_utils.run_bass_kernel_spmd(
        nc, in_maps, core_ids=list(range(NCORES)), trace=trace,
        **trace_kwargs)
    return _gather(res.results), res


def kernel(x, W1, b1, W2, b2):
    out, _ = run(x, W1, b1, W2, b2)
    return out


# revision 29
# speedup vs baseline: 1.1693x; 1.0177x over previous
"""Trainium2 Bass kernel for a binarized-weight MLP (BNN MNIST-style):

    h   = x @ sign(W1).T + b1      # fc1, binarized weights
    h   = clip(h, -1, 1)           # Hardtanh
    out = h @ W2.T + b2            # fc2

Shapes: x [8192, 784] f32, W1 [4096, 784], b1 [4096], W2 [10, 4096], b2 [10].

Strategy (data-parallel over 8 NeuronCores):
  - Shard batch 8192 -> 1024 rows/core; replicate weights. All matmuls bf16
    (sign(W1) exact in bf16), fp32 PSUM.
  - fc1 contraction K = 784 x-rows + 2 bias rows (b1 hi/lo bf16 split with
    ones-columns in x) = 786 rows = 6 full 128-row k-tiles + an 18-row tail.
  - The 18-row tail is ROW-TILED: hidden tiles are processed in quads of 4,
    and one PE slot runs 4 concurrent K=18 matmuls at tile_position
    (0/32/64/96, 0) - one per ht in the quad - so the tail costs 1 slot per
    quad instead of 4.  Slot order per quad: packed tail first (start=True
    for all 4 PSUM groups), then ht-by-ht k0..5 with stop at k5, so each
    ht's Hardtanh (DVE tensor_scalar min/max) pipelines inside the quad.
  - fc2 (M=10) is COL-TILED: per quad one burst of 4 concurrent matmuls at
    tile_position (0, 0/32/64/96) accumulates W2-slices @ h into partition
    slices 32r..32r+9 of a per-nt PSUM tile; bursts are issued one quad
    behind fc1.  The 4 partition groups are reduced by one selector matmul
    (sel[32r+o, o] = 1) over an ACT-engine copy of the PSUM tile, with b2
    folded into the selector via a ones-row at partition 127.
  - 4 PE warm-up matmuls bridge the DMA head so the HAM clock gate
    un-throttles (1.2 -> 2.4 GHz) as early as possible.
  - Per-core output is out^T [10, 1024] f32; host gathers + transposes.
"""

import numpy as np
import ml_dtypes
from contextlib import ExitStack

import concourse.bass as bass
import concourse.mybir as mybir
import concourse.tile as tile
from concourse import bacc
from concourse import bass_utils

BF16_NP = ml_dtypes.bfloat16
FP8_NP = ml_dtypes.float8_e4m3
BF16 = mybir.dt.bfloat16
FP8 = mybir.dt.float8e4
F32 = mybir.dt.float32

BATCH, IN, HID, OUT = 8192, 784, 4096, 10
NCORES = 8
B_CORE = BATCH // NCORES        # 1024
NT = B_CORE // 512              # 2 batch n-tiles of 512 per core
HT = HID // 128                 # 32 hidden tiles
NQ = HT // 4                    # 8 quads of 4 hidden tiles
KF = 6                          # full 128-row k-tiles (768 rows)
KTAIL = IN + 2 - KF * 128       # 18 tail rows (16 x-rows + 2 bias rows)
N_WARMUP = 4                    # PE warm-up matmuls (HAM un-throttle)

_CACHE = {}


def _build():
    """Build + compile the Bacc graph once per process."""
    if "nc" in _CACHE:
        return _CACHE["nc"]

    nc = bacc.Bacc("TRN2", target_bir_lowering=False, debug=False,
                   num_devices=NCORES)
    xt_d = nc.dram_tensor("xt", [NT, 128, KF * 512], BF16,
                          kind="ExternalInput").ap()
    xt6_d = nc.dram_tensor("xt6", [NT, 128, 512], BF16,
                           kind="ExternalInput").ap()
    # sign(W1) in {-1,0,+1} is exact in fp8 (the moving tensor stays bf16,
    # which sets the matmul rate) — halves the dominant DMA stream.
    w1_d = nc.dram_tensor("w1", [HT // 2, 128, 2 * KF * 128], FP8,
                          kind="ExternalInput").ap()
    w1k7_d = nc.dram_tensor("w1k7", [128, NQ * 128], FP8,
                            kind="ExternalInput").ap()
    w2_d = nc.dram_tensor("w2", [128, HT * OUT], BF16,
                          kind="ExternalInput").ap()
    sel_d = nc.dram_tensor("sel", [128, OUT], BF16,
                           kind="ExternalInput").ap()
    out_d = nc.dram_tensor("out", [OUT, B_CORE], F32,
                           kind="ExternalOutput").ap()

    # Raw SBUF tensor for PE warm-up matmuls: contents irrelevant, results
    # discarded into a scratch PSUM bank that is later reused by the
    # selector matmul.
    warm_sb = nc.alloc_sbuf_tensor("warm_raw", [128, 512], BF16).ap()

    with tile.TileContext(nc) as tc:
        with ExitStack() as ctx:
            wpool = ctx.enter_context(tc.tile_pool(name="w1", bufs=1))
            xpool = ctx.enter_context(tc.tile_pool(name="x", bufs=1))
            cpool = ctx.enter_context(tc.tile_pool(name="const", bufs=1))
            # h: single rotating tag; 16 bufs >> the ~13-alloc read distance
            # (fc2 bursts cover the 2 oldest of up to 3 pending quads)
            hpool = ctx.enter_context(tc.tile_pool(name="h", bufs=16))
            s2pool = ctx.enter_context(tc.tile_pool(name="s2", bufs=1))
            # PSUM bank budget (8 banks of [128,512]f32):
            #   ps1 x6 (rotating; 6 > 4/quad + 1 so a quad's packed matmuls
            #   never WAR-wait on the previous quad's in-flight Hardtanh
            #   read of the same bank) + ps2 x1 (shared by both nt; nt0's
            #   is ACT-drained before nt1's first deferred burst) +
            #   psel/warm x1
            ps1pool = ctx.enter_context(
                tc.tile_pool(name="ps1", bufs=6, space="PSUM"))
            ps2pool = ctx.enter_context(
                tc.tile_pool(name="ps2", bufs=1, space="PSUM"))
            pselpool = ctx.enter_context(
                tc.tile_pool(name="psel", bufs=1, space="PSUM"))

            w1k7_sb = cpool.tile([128, NQ * 128], FP8, tag="w1k7")
            w2_sb = cpool.tile([128, HT * OUT], BF16, tag="w2")
            sel_sb = cpool.tile([128, OUT], BF16, tag="sel")
            out_sb = cpool.tile([OUT, B_CORE], F32, tag="out")
            xt6_sb = [cpool.tile([128, 512], BF16, tag=f"xt6_{nt}",
                                 name=f"xt6_{nt}")
                      for nt in range(NT)]

            # PE warm-up: HAM keeps the PE at 1.2 GHz until ~3.4us of
            # sustained matmul activity; run dummy matmuls (no deps, start
            # immediately) while the first input DMAs stream.  They share
            # the psel PSUM bank (long free before the selector needs it).
            for i in range(N_WARMUP):
                pw = pselpool.tile([128, 512], F32, tag="psel",
                                   name=f"pswarm_{i}")
                nc.tensor.matmul(pw[:], warm_sb[:, 0:128], warm_sb[:],
                                 start=True, stop=True)

            # Input DMAs on two parallel HWDGE queues, ordered so the
            # first-consumed tiles land first:
            #   sync:   w1k7, w1[0..31]
            #   scalar: xt6[0], x(0,k*), w2, sel, xt6[1], x(1,k*)
            # w1 in 2-ht pairs (halves DMA-issue count); consumers slice.
            w1_p = [wpool.tile([128, 2 * KF * 128], FP8, tag=f"w1_{hp}",
                               name=f"w1_{hp}")
                    for hp in range(HT // 2)]

            def w1_slice(ht, kt):
                base = kt * 256 + (ht % 2) * 128
                return w1_p[ht // 2][:, base:base + 128]

            # x as one wide tile per nt; per-kt column-slice DMAs so the
            # consumers' arrival pacing is fine-grained.
            x_w = [xpool.tile([128, KF * 512], BF16, tag=f"x_{nt}",
                              name=f"x_{nt}")
                   for nt in range(NT)]

            def x_slice(nt, kt):
                return x_w[nt][:, kt * 512:(kt + 1) * 512]

            def x_dma(eng, nt, kt):
                eng.dma_start(x_slice(nt, kt), xt_d[nt, :,
                                                    kt * 512:(kt + 1) * 512])

            # Three parallel DMA queues, each carrying part of quad 0's
            # critical prefix first (the early DMA rate is the head's
            # bottleneck), then streaming the rest.
            nc.sync.dma_start(w1k7_sb[:, 0:128], w1k7_d[:, 0:128])
            nc.sync.dma_start(w1_p[0][:], w1_d[0])
            for kt in (4, 5):
                x_dma(nc.sync, 0, kt)
            nc.sync.dma_start(w1k7_sb[:, 128:NQ * 128],
                              w1k7_d[:, 128:NQ * 128])
            for hp in range(1, HT // 2):
                nc.sync.dma_start(w1_p[hp][:], w1_d[hp])

            nc.gpsimd.dma_start(xt6_sb[0][:], xt6_d[0])
            for kt in (0, 1):
                x_dma(nc.gpsimd, 0, kt)

            for kt in (2, 3):
                x_dma(nc.scalar, 0, kt)
            nc.scalar.dma_start(w2_sb[:], w2_d)
            nc.scalar.dma_start(sel_sb[:], sel_d)
            nc.scalar.dma_start(xt6_sb[1][:], xt6_d[1])
            for kt in range(KF):
                x_dma(nc.scalar, 1, kt)

            # deferred actions, emitted between fc1 ht-groups so PE-queue
            # stalls on not-yet-ready DVE/ACT results are avoided
            deferred = []

            def sel_chain(nt):
                # ps2 partition groups {0,32,64,96}+0..9 -> out via one
                # selector matmul; b2 rides on sel partition 127 against the
                # ones rows of s2 (pre-memset to 1.0 at kernel start).
                s2 = s2_sb[nt]
                psel = pselpool.tile([128, 512], F32, tag="psel",
                                     name=f"psel_{nt}")

                def copy():
                    nc.scalar.activation(
                        s2[0:106, :], ps2[0:106, :],
                        mybir.ActivationFunctionType.Identity)

                def mm():
                    nc.tensor.matmul(psel[0:OUT, :], sel_sb[:], s2[:],
                                     start=True, stop=True,
                                     skip_group_check=True)

                def out():
                    nc.scalar.activation(
                        out_sb[:, nt * 512:(nt + 1) * 512], psel[0:OUT, :],
                        mybir.ActivationFunctionType.Identity)
                    nc.sync.dma_start(
                        out_d[:, nt * 512:(nt + 1) * 512],
                        out_sb[:, nt * 512:(nt + 1) * 512])

                def noop():
                    pass
                return [copy, noop, mm, out]

            ps2 = ps2pool.tile([128, 512], F32, tag="ps2", name="ps2")
            s2_sb = [s2pool.tile([128, 512], BF16, tag=f"s2_{nt}",
                                 name=f"s2_{nt}")
                     for nt in range(NT)]
            # fc2 only ever writes partition groups 32r..32r+9 of ps2, and
            # the ACT copy in sel_chain reads ps2[0:106) into s2 whose rows
            # 106..127 must be 1.0 (bias row) — stale PSUM/SBUF could hold
            # NaN and NaN*0 = NaN in the selector matmul.  Full-tile
            # memsets at kernel start (engines idle; start=True matmuls
            # overwrite their elements regardless).
            nc.vector.memset(ps2[:], 0.0)
            for nt in range(NT):
                nc.gpsimd.memset(s2_sb[nt][:], 1.0)

            # fc2 col-tiled matmuls, batched and emitted right after a
            # packed slot (all PE array-mask switches grouped at the quad
            # boundary).  Each burst runs 4 col groups concurrently; within
            # a group the quads serialize.  Only the OLDEST quads are
            # flushed — the newest quad's Hardtanh results are still in
            # flight on the DVE at flush time.
            pending_fc2 = []

            def flush_fc2(n):
                batch, pending_fc2[:] = pending_fc2[:n], pending_fc2[n:]
                for r in range(4):
                    for q, hs in batch:
                        nc.tensor.matmul(
                            ps2[32 * r:32 * r + OUT, :],
                            w2_sb[:, (4 * q + r) * OUT:
                                  (4 * q + r + 1) * OUT], hs[r][:],
                            start=(q == 0), stop=(q == NQ - 1),
                            tile_position=(0, 32 * r),
                            skip_group_check=True)

            for nt in range(NT):
                for q in range(NQ):
                    ps1 = [ps1pool.tile([128, 512], F32, tag="ps1",
                                        name=f"ps1_{nt}_{q}_{r}")
                           for r in range(4)]
                    # packed 18-row tail: 4 concurrent row-tiled matmuls
                    for r in range(4):
                        nc.tensor.matmul(
                            ps1[r][:],
                            w1k7_sb[32 * r:32 * r + KTAIL,
                                    q * 128:(q + 1) * 128],
                            xt6_sb[nt][32 * r:32 * r + KTAIL, :],
                            start=True, stop=False,
                            tile_position=(32 * r, 0),
                            skip_group_check=True)
                    # flush the 2 oldest pending quads (their Hardtanhs are
                    # long done); at an nt boundary flush everything so the
                    # sel-chain ACT copy (popped below) sees all 8 quads
                    if q == 0:
                        flush_fc2(len(pending_fc2))
                    elif len(pending_fc2) >= 3:
                        flush_fc2(2)
                    hs = []
                    for r in range(4):
                        ht = 4 * q + r
                        for kt in range(KF):
                            nc.tensor.matmul(
                                ps1[r][:],
                                w1_slice(ht, kt),
                                x_slice(nt, kt),
                                start=False, stop=(kt == KF - 1),
                                skip_group_check=True)
                        h = hpool.tile([128, 512], BF16, tag="h",
                                       name=f"h_{nt}_{ht}")
                        # Hardtanh + downcast: h = max(min(ps1, 1), -1)
                        nc.vector.tensor_scalar(
                            h[:], ps1[r][:], 1.0, -1.0,
                            op0=mybir.AluOpType.min,
                            op1=mybir.AluOpType.max)
                        hs.append(h)
                        # emit one deferred action after each ht group
                        if deferred:
                            deferred.pop(0)()
                    pending_fc2.append((q, hs))
                    if q == NQ - 1:
                        deferred.extend(sel_chain(nt))
            flush_fc2(len(pending_fc2))
            for fn in deferred:
                fn()

    nc.compile()
    _CACHE["nc"] = nc
    return nc


def _prep_inputs(x, W1, b1, W2, b2):
    """Host-side shard + layout prep. Returns in_maps for the 8 cores."""
    x = np.asarray(x, dtype=np.float32)
    W1 = np.asarray(W1, dtype=np.float32)
    b1 = np.asarray(b1, dtype=np.float32)
    W2 = np.asarray(W2, dtype=np.float32)
    b2 = np.asarray(b2, dtype=np.float32)

    K = IN + 2                                           # 786
    # fc1 weight in fp8 (sign values exact), augmented with two bias rows
    # (hi + lo fp8 split of b1; residual ~0.4% of b1, negligible vs the
    # Hardtanh clip scale).
    w1aug = np.zeros((K, HID), dtype=np.float32)
    w1aug[:IN] = np.sign(W1).T
    b1_hi = b1.astype(FP8_NP).astype(np.float32)
    w1aug[IN] = b1_hi
    w1aug[IN + 1] = b1 - b1_hi
    w1aug = w1aug.astype(FP8_NP)

    # full k-tiles in 2-ht pairs: [hp, p, kt*256 + j*128 + m]
    #   = w1aug[kt*128+p, (2hp+j)*128+m]
    w1_host = np.ascontiguousarray(
        w1aug[:KF * 128].reshape(KF, 128, HT // 2, 2 * 128)
        .transpose(2, 1, 0, 3).reshape(HT // 2, 128, 2 * KF * 128))

    # 18-row tail, packed for 4-way row tiling:
    # [32r+j, q*128+m] = w1aug[768+j, (4q+r)*128+m]
    w1k7 = np.zeros((128, NQ * 128), dtype=FP8_NP)
    tail = w1aug[KF * 128:].reshape(KTAIL, NQ, 4, 128)   # [j, q, r, m]
    for r in range(4):
        w1k7[32 * r:32 * r + KTAIL] = tail[:, :, r, :].reshape(KTAIL, -1)

    # fc2 weight: [p, ht*10+o] = W2[o, ht*128+p]
    w2_host = np.ascontiguousarray(
        W2.T.astype(BF16_NP).reshape(HT, 128, OUT)
        .transpose(1, 0, 2).reshape(128, HT * OUT))

    # selector for the 4-way fc2 partition-group reduction, b2 on row 127
    sel_host = np.zeros((128, OUT), dtype=BF16_NP)
    for r in range(4):
        for o in range(OUT):
            sel_host[32 * r + o, o] = 1
    sel_host[127, :] = b2.astype(BF16_NP)

    # x augmented with ones-columns matching the two b1 rows.
    x_aug = np.zeros((BATCH, K), dtype=BF16_NP)
    x_aug[:, :IN] = x.astype(BF16_NP)
    x_aug[:, IN] = 1
    x_aug[:, IN + 1] = 1

    in_maps = []
    for c in range(NCORES):
        xc = x_aug[c * B_CORE:(c + 1) * B_CORE]          # [1024, 786]
        # full k-tiles: [nt, p, kt*512+b] = xc[nt*512+b, kt*128+p]
        xt = np.ascontiguousarray(
            xc[:, :KF * 128].reshape(NT, 512, KF, 128)
            .transpose(0, 3, 2, 1).reshape(NT, 128, KF * 512))
        # tail block replicated at partition bases 0/32/64/96
        xt6 = np.zeros((NT, 128, 512), dtype=BF16_NP)
        tail_x = (xc[:, KF * 128:].reshape(NT, 512, KTAIL)
                  .transpose(0, 2, 1))                   # [nt, j, b]
        for r in range(4):
            xt6[:, 32 * r:32 * r + KTAIL, :] = tail_x
        in_maps.append({"xt": xt, "xt6": xt6, "w1": w1_host,
                        "w1k7": w1k7, "w2": w2_host, "sel": sel_host})
    return in_maps


def _gather(results):
    full = np.concatenate([np.asarray(r["out"], dtype=np.float32)
                           for r in results], axis=1)    # [10, 8192]
    return np.ascontiguousarray(full.T)                  # [8192, 10]


def run(x, W1, b1, W2, b2, trace=False, **trace_kwargs):
    import os
    nc = _build()
    in_maps = _prep_inputs(x, W1, b1, W2, b2)
    if not trace:
        # The NTFF profiling hook isn't available in every environment;
        # make sure an ambient BASS_TRACE can't pull us onto that path.
        os.environ["BASS_NEVER_TRACE"] = "1"
    else:
        os.environ.pop("BASS_NEVER_TRACE", None)
    res = bass_utils.run_bass_kernel_spmd(
        nc, in_maps, core_ids=list(range(NCORES)), trace=trace,
        **trace_kwargs)
    return _gather(res.results), res


def kernel(x, W1, b1, W2, b2):
    out, _ = run(x, W1, b1, W2, b2)
    return out
